# revision 1
# baseline (speedup 1.0000x reference)
"""Trainium2 Bass kernel for the CT-metrics pairwise MLP scorer.

Computes, for M_aug [N,D] and Q [M,D] (N=M=512, D=1024):
    diff2[n,m,:] = (M_aug[n]-Q[m])**2
    cost[n,m]    = diff2.sum(-1)
    d[n,m]       = -(MLP(diff2[n,m,:]) + b3)      (D->512->256->1, leaky relu)
    bw           = softmax(d, axis=0)
    score[m]     = sum_n cost*bw,  score_fg[m] = sum_{n<N_fg} cost*bw

Sharding: N axis split across 8 cores (64 rows each). Each core computes
partial column sums S1 = sum_n exp(d), S1fg, Sc' = sum_n exp(d)*(||Mn||^2
- 2 M.Q^T) and Scfg' (flash-softmax style; logits are O(1) so no max shift
is needed).  Host combine:  score = Sc'/S1 + ||Q||^2,
score_fg = Scfg'/S1 + ||Q||^2 * S1fg/S1.

Device layout: contraction dims sit on SBUF partitions.  Per local row n:
diff2^T[d, m] chunk tiles are produced on THREE engines (ACT fused
square-with-bias; DVE and GPSIMD as add-broadcast + self-multiply) so no
single elementwise engine bottlenecks; layers 1/2/3 are bf16 matmuls
(fp32 PSUM accumulate); layer 3 uses h2 slices as stationary so the
logits land transposed as [m_chunk, n] in PSUM, giving free-dim softmax
reductions.  Inputs arrive packed in two DRAM tensors (one f32, one bf16)
so the whole load is 2 DMAs; output per core is a [128, 4, 4] stats tile.
"""

from contextlib import ExitStack

import numpy as np

import concourse.bass as bass
import concourse.tile as tile
from concourse import bacc, mybir
from concourse.bass_utils import run_bass_kernel_spmd

N_CORES = 8
N, M, D, H = 512, 512, 1024, 512
K2 = H // 2  # 256
NL = N // N_CORES  # 64 rows per core
DC, HC, KC, MC = D // 128, H // 128, K2 // 128, M // 128  # 8, 4, 2, 4
NEG_SLOPE = 0.01

F32 = mybir.dt.float32
BF16 = mybir.dt.bfloat16
AF = mybir.ActivationFunctionType

# d2 chunk producer per dc index: 'a' = ACT fused square,
# 'v' = DVE 2-step, 'p' = GPSIMD 2-step
D2_SPLIT = "aaaaaaaa"

# packed f32 column offsets
_QT0 = 0
_MT0 = _QT0 + DC * M          # 4096
_B10 = _MT0 + DC * NL         # 4608
_B20 = _B10 + HC              # 4612
_B30 = _B20 + KC              # 4614
_MN0 = _B30 + 1               # 4615  (0.5*||Mn||^2 row, partition 0)
_FG0 = _MN0 + NL              # 4679  (fg mask row tiled MC times, part 0)
_ON0 = _FG0 + MC * NL         # 4935  (ones row, partition 0)
_PF_COLS = _ON0 + 128         # 5063

# packed bf16 column offsets
_W10 = 0
_W20 = _W10 + DC * H          # 4096
_W30 = _W20 + HC * K2         # 5120
_PB_COLS = _W30 + KC          # 5122


def emit_body(nc, tc, ctx, pf_sb, pb_sb, stats, act_fn, d2_split=D2_SPLIT,
              pools=None, fake_d2=False, batch2=False):
    """Emit prologue + n-loop + epilogue reading packed SBUF tiles."""
    if pools is None:
        pools = {}

    def pool(name, bufs, space="SBUF"):
        if name not in pools:
            pools[name] = ctx.enter_context(
                tc.tile_pool(name=name, bufs=bufs, space=space))
        return pools[name]

    consts = pool("consts", 1)
    diffp = pool("diffp", 2)
    h1p = pool("h1p", 2)
    h2p = pool("h2p", 2)
    ep = pool("ep", 1)
    tvp = pool("tvp", 2)
    tpp = pool("tpp", 2)
    l1ps = pool("l1ps", 3 if batch2 else 5, "PSUM")
    l2ps = pool("l2ps", 2 if batch2 else 1, "PSUM")
    l3ps = pool("l3ps", 1, "PSUM")

    qt_sb = pf_sb[:, _QT0:_MT0].rearrange("p (c m) -> p c m", c=DC)
    mt_sb = pf_sb[:, _MT0:_B10].rearrange("p (c n) -> p c n", c=DC)
    b1_sb = pf_sb[:, _B10:_B20]
    b2_sb = pf_sb[:, _B20:_B30]
    b3_sb = pf_sb[:, _B30:_B30 + 1]
    mn2h = pf_sb[0:1, _MN0:_MN0 + NL]
    fg_row = pf_sb[0:1, _FG0:_FG0 + MC * NL]
    onesr = pf_sb[0:1, _ON0:_ON0 + 128]
    w1_sb = pb_sb[:, _W10:_W20].rearrange("p (c h) -> p c h", c=DC)
    w2_sb = pb_sb[:, _W20:_W30].rearrange("p (c k) -> p c k", c=HC)
    w3_sb = pb_sb[:, _W30:_W30 + KC].rearrange("p (c o) -> p c o", c=KC)

    # ---- prologue (all deps resolve to the two input DMAs) ----
    g_t = l3ps.tile([128, MC, NL], F32, tag="ps3")
    for mc in range(MC):
        for dc in range(DC):
            nc.tensor.matmul(g_t[:, mc, :],
                             qt_sb[:, dc, mc * 128:(mc + 1) * 128],
                             mt_sb[:, dc, :],
                             start=(dc == 0), stop=False)
        nc.tensor.matmul(g_t[:, mc, :], onesr, mn2h, start=False, stop=True)
    cost_t = consts.tile([128, MC, NL], F32, tag="cost_t")
    nc.vector.tensor_scalar_mul(cost_t[:], g_t[:], 2.0)

    mask_ps = l1ps.tile([128, MC * NL], F32, tag="ps1")
    nc.tensor.matmul(mask_ps[:], onesr, fg_row, start=True, stop=True)
    mask_bc = consts.tile([128, MC, NL], F32, tag="mask_bc")
    nc.vector.tensor_copy(mask_bc[:], mask_ps[:].rearrange(
        "p (c n) -> p c n", c=MC))

    d_ps = l3ps.tile([128, MC, NL], F32, tag="ps3")

    def produce_d2(n):
        if fake_d2:
            return pb_sb[:, 0:DC * M].rearrange("p (c m) -> p c m", c=DC)
        d2 = diffp.tile([128, DC, M], BF16, tag="d2", name="d2")
        for dc in range(DC):
            eng = d2_split[dc]
            if eng == "a":
                nc.scalar.activation(d2[:, dc, :], qt_sb[:, dc, :], AF.Square,
                                     bias=mt_sb[:, dc, n:n + 1])
            elif eng == "v":
                tv = tvp.tile([128, M], BF16, tag="tv", name="tv")
                nc.vector.tensor_scalar_add(tv[:], qt_sb[:, dc, :],
                                            mt_sb[:, dc, n:n + 1])
                nc.vector.tensor_mul(d2[:, dc, :], tv[:], tv[:])
            else:
                tp = tpp.tile([128, M], BF16, tag="tp", name="tp")
                nc.gpsimd.tensor_scalar_add(tp[:], qt_sb[:, dc, :],
                                            mt_sb[:, dc, n:n + 1])
                nc.gpsimd.tensor_mul(d2[:, dc, :], tp[:], tp[:])
        return d2

    def l3_emit(n, h2):
        for mc in range(MC):
            for kc in range(KC):
                nc.tensor.matmul(
                    d_ps[:, mc, n:n + 1],
                    h2[:, kc, mc * 128:(mc + 1) * 128],
                    w3_sb[:, kc, :],
                    start=(kc == 0), stop=(kc == KC - 1))

    if batch2:
        for n in range(0, NL, 2):
            d2a = produce_d2(n)
            d2b = produce_d2(n + 1)
            h1a = h1p.tile([128, HC, M], BF16, tag="h1", name="h1a")
            h1b = h1p.tile([128, HC, M], BF16, tag="h1", name="h1b")
            for hc in range(HC):
                p1a = l1ps.tile([128, M], F32, tag="ps1", name="p1a")
                p1b = l1ps.tile([128, M], F32, tag="ps1", name="p1b")
                for dc in range(DC):
                    w_slice = w1_sb[:, dc, hc * 128:(hc + 1) * 128]
                    nc.tensor.matmul(p1a[:], w_slice, d2a[:, dc, :],
                                     start=(dc == 0), stop=(dc == DC - 1))
                    nc.tensor.matmul(p1b[:], w_slice, d2b[:, dc, :],
                                     start=(dc == 0), stop=(dc == DC - 1))
                nc.scalar.activation(h1a[:, hc, :], p1a[:], act_fn,
                                     bias=b1_sb[:, hc:hc + 1],
                                     alpha=NEG_SLOPE)
                nc.scalar.activation(h1b[:, hc, :], p1b[:], act_fn,
                                     bias=b1_sb[:, hc:hc + 1],
                                     alpha=NEG_SLOPE)
            p2a = l2ps.tile([128, KC, M], F32, tag="ps2", name="p2a")
            p2b = l2ps.tile([128, KC, M], F32, tag="ps2", name="p2b")
            for kc in range(KC):
                for hc in range(HC):
                    w_slice = w2_sb[:, hc, kc * 128:(kc + 1) * 128]
                    nc.tensor.matmul(p2a[:, kc, :], w_slice, h1a[:, hc, :],
                                     start=(hc == 0), stop=(hc == HC - 1))
                    nc.tensor.matmul(p2b[:, kc, :], w_slice, h1b[:, hc, :],
                                     start=(hc == 0), stop=(hc == HC - 1))
            h2a = h2p.tile([128, KC, M], BF16, tag="h2", name="h2a")
            h2b = h2p.tile([128, KC, M], BF16, tag="h2", name="h2b")
            for kc in range(KC):
                nc.scalar.activation(h2a[:, kc, :], p2a[:, kc, :], act_fn,
                                     bias=b2_sb[:, kc:kc + 1],
                                     alpha=NEG_SLOPE)
                nc.scalar.activation(h2b[:, kc, :], p2b[:, kc, :], act_fn,
                                     bias=b2_sb[:, kc:kc + 1],
                                     alpha=NEG_SLOPE)
            l3_emit(n, h2a)
            l3_emit(n + 1, h2b)
    else:
        _unused = 0
    # ---- main loop over local rows ----
    for n in range(NL if not batch2 else 0):
        if fake_d2:
            d2 = pb_sb[:, 0:DC * M].rearrange("p (c m) -> p c m", c=DC)
        else:
            d2 = diffp.tile([128, DC, M], BF16, tag="d2")
        for dc in range(DC if not fake_d2 else 0):
            eng = d2_split[dc]
            if eng == "a":
                nc.scalar.activation(d2[:, dc, :], qt_sb[:, dc, :], AF.Square,
                                     bias=mt_sb[:, dc, n:n + 1])
            elif eng == "v":
                tv = tvp.tile([128, M], BF16, tag="tv")
                nc.vector.tensor_scalar_add(tv[:], qt_sb[:, dc, :],
                                            mt_sb[:, dc, n:n + 1])
                nc.vector.tensor_mul(d2[:, dc, :], tv[:], tv[:])
            else:
                tp = tpp.tile([128, M], BF16, tag="tp")
                nc.gpsimd.tensor_scalar_add(tp[:], qt_sb[:, dc, :],
                                            mt_sb[:, dc, n:n + 1])
                nc.gpsimd.tensor_mul(d2[:, dc, :], tp[:], tp[:])

        h1 = h1p.tile([128, HC, M], BF16, tag="h1")
        for hc in range(HC):
            p1 = l1ps.tile([128, M], F32, tag="ps1")
            for dc in range(DC):
                nc.tensor.matmul(
                    p1[:],
                    w1_sb[:, dc, hc * 128:(hc + 1) * 128],
                    d2[:, dc, :],
                    start=(dc == 0), stop=(dc == DC - 1))
            nc.scalar.activation(h1[:, hc, :], p1[:], act_fn,
                                 bias=b1_sb[:, hc:hc + 1], alpha=NEG_SLOPE)

        p2 = l2ps.tile([128, KC, M], F32, tag="ps2")
        for kc in range(KC):
            for hc in range(HC):
                nc.tensor.matmul(
                    p2[:, kc, :],
                    w2_sb[:, hc, kc * 128:(kc + 1) * 128],
                    h1[:, hc, :],
                    start=(hc == 0), stop=(hc == HC - 1))
        h2 = h2p.tile([128, KC, M], BF16, tag="h2")
        for kc in range(KC):
            nc.scalar.activation(h2[:, kc, :], p2[:, kc, :], act_fn,
                                 bias=b2_sb[:, kc:kc + 1], alpha=NEG_SLOPE)

        for mc in range(MC):
            for kc in range(KC):
                nc.tensor.matmul(
                    d_ps[:, mc, n:n + 1],
                    h2[:, kc, mc * 128:(mc + 1) * 128],
                    w3_sb[:, kc, :],
                    start=(kc == 0), stop=(kc == KC - 1))

    # ---- epilogue ----
    e_t = ep.tile([128, MC, NL], F32, tag="e_t")
    nc.scalar.activation(e_t[:], d_ps[:], AF.Exp, bias=b3_sb, scale=-1.0)
    w_t = ep.tile([128, MC, NL], F32, tag="w_t")
    nc.vector.tensor_mul(w_t[:], e_t[:], cost_t[:])
    efg_t = ep.tile([128, MC, NL], F32, tag="efg_t")
    nc.vector.tensor_mul(efg_t[:], e_t[:], mask_bc[:])
    wfg_t = ep.tile([128, MC, NL], F32, tag="wfg_t")
    nc.vector.tensor_mul(wfg_t[:], w_t[:], mask_bc[:])

    stats_sb = consts.tile([128, 4, MC], F32, tag="stats_sb")
    for s, src in ((0, e_t), (1, efg_t), (2, w_t), (3, wfg_t)):
        nc.vector.tensor_reduce(stats_sb[:, s, :], src[:],
                                axis=mybir.AxisListType.X,
                                op=mybir.AluOpType.add)
    nc.sync.dma_start(stats[:], stats_sb[:])


def build_program(act_fn=AF.Lrelu, d2_split=D2_SPLIT):
    nc = bacc.Bacc("TRN2", target_bir_lowering=False, debug=False,
                   num_devices=N_CORES)
    pf = nc.dram_tensor("pf", [128, _PF_COLS], F32, kind="ExternalInput").ap()
    pb = nc.dram_tensor("pb", [128, _PB_COLS], BF16, kind="ExternalInput").ap()
    stats = nc.dram_tensor("stats", [128, 4, MC], F32,
                           kind="ExternalOutput").ap()

    with tile.TileContext(nc) as tc, ExitStack() as ctx:
        consts = ctx.enter_context(tc.tile_pool(name="consts", bufs=1))
        pf_sb = consts.tile([128, _PF_COLS], F32, tag="pf_sb")
        nc.sync.dma_start(pf_sb[:], pf[:])
        pb_sb = consts.tile([128, _PB_COLS], BF16, tag="pb_sb")
        nc.sync.dma_start(pb_sb[:], pb[:])
        emit_body(nc, tc, ctx, pf_sb, pb_sb, stats, act_fn, d2_split,
                  pools={"consts": consts})

    nc.compile()
    return nc


def shard_inputs(M_aug, Q, W1, b1, W2, b2, W3, b3, N_fg):
    """Host-side layout prep. Returns per-core input maps."""
    import ml_dtypes
    f = np.float32
    bf = ml_dtypes.bfloat16
    M_aug = np.asarray(M_aug, f)
    Q = np.asarray(Q, f)
    W1 = np.asarray(W1, f)
    W2 = np.asarray(W2, f)
    W3 = np.asarray(W3, f)
    b1 = np.asarray(b1, f)
    b2 = np.asarray(b2, f)
    b3 = np.asarray(b3, f)
    nfg = int(N_fg)

    def part_major(a2d, chunks):  # [C*128, F] -> [128, C*F]
        cdim, fdim = a2d.shape
        assert cdim == chunks * 128
        return np.ascontiguousarray(
            a2d.reshape(chunks, 128, fdim).transpose(1, 0, 2)).reshape(128, -1)

    pb_v = np.zeros((128, _PB_COLS), bf)
    pb_v[:, _W10:_W20] = part_major(W1.T, DC).astype(bf)
    pb_v[:, _W20:_W30] = part_major(W2.T, HC).astype(bf)
    pb_v[:, _W30:_W30 + KC] = part_major(W3.reshape(K2, 1), KC).astype(bf)
    pb_v = np.ascontiguousarray(pb_v)

    base = np.zeros((128, _PF_COLS), f)
    base[:, _QT0:_MT0] = part_major(Q.T, DC)
    base[:, _B10:_B20] = b1.reshape(HC, 128).T
    base[:, _B20:_B30] = b2.reshape(KC, 128).T
    base[:, _B30] = -float(b3[0])
    base[0, _ON0:_ON0 + 128] = 1.0

    in_maps = []
    for c in range(N_CORES):
        rows = slice(c * NL, (c + 1) * NL)
        pf_v = base.copy()
        pf_v[:, _MT0:_B10] = part_major(-M_aug[rows].T, DC)
        pf_v[0, _MN0:_MN0 + NL] = 0.5 * (M_aug[rows] ** 2).sum(-1)
        gidx = np.arange(c * NL, (c + 1) * NL)
        pf_v[0, _FG0:_FG0 + MC * NL] = np.tile((gidx < nfg).astype(f), MC)
        in_maps.append({"pf": pf_v, "pb": pb_v})
    return in_maps


def combine(stats_list, Q):
    """stats_list: per-core [128, 4, MC] arrays -> (score, score_fg)."""
    st = np.stack([
        np.asarray(s, np.float64).transpose(1, 2, 0).reshape(4, M)
        for s in stats_list
    ])  # [C, 4, M]
    S1 = st[:, 0].sum(0)
    S1fg = st[:, 1].sum(0)
    Sc = st[:, 2].sum(0)
    Scfg = st[:, 3].sum(0)
    qn2 = (np.asarray(Q, np.float64) ** 2).sum(-1)
    score = Sc / S1 + qn2
    score_fg = Scfg / S1 + qn2 * (S1fg / S1)
    return score.astype(np.float32), score_fg.astype(np.float32)


_PROGRAM_CACHE = {}


def run(trace=False, **inputs):
    if "prog" not in _PROGRAM_CACHE:
        _PROGRAM_CACHE["prog"] = build_program()
    nc = _PROGRAM_CACHE["prog"]
    in_maps = shard_inputs(**inputs)
    res = run_bass_kernel_spmd(nc, in_maps, list(range(N_CORES)), trace=trace)
    outs = combine([res.results[c]["stats"] for c in range(N_CORES)],
                   inputs["Q"])
    return outs, res


def kernel(**inputs):
    outs, _ = run(trace=False, **inputs)
    return outs



# revision 5
# speedup vs baseline: 2.1487x; 2.1487x over previous
"""Trainium2 Bass kernel for the CT-metrics pairwise MLP scorer (fp8 DoubleRow).

Computes, for M_aug [N,D] and Q [M,D] (N=M=512, D=1024):
    diff2[n,m,:] = (M_aug[n]-Q[m])**2
    cost[n,m]    = diff2.sum(-1)
    d[n,m]       = -(MLP(diff2[n,m,:]) + b3)      (D->512->256->1, leaky relu)
    bw           = softmax(d, axis=0)
    score[m]     = sum_n cost*bw,  score_fg[m] = sum_{n<N_fg} cost*bw

Sharding: N axis split across 8 cores (64 rows each).  Per-core partial
column sums S1 = sum_n exp(d), S1fg, Sc = sum_n exp(d)*(||Mn||^2 - 2 Mn.Q)
and Scfg; host combine adds ||Q||^2 and normalizes (flash-softmax style,
logits are O(1) so no max shift needed).

Speed strategy vs the bf16 baseline: all three MLP matmuls run as fp8e4m3
DoubleRow matmuls (2 contraction chunks per instruction at 0.5 cyc/row =
4x bf16 FLOP rate).  The layer-1 moving operand is built per row with ONE
elementwise op per 128-chunk using the identity
    W1 . diff2  =  (2*W1) . X + [W1 . Mn^2]     X = qt*mt + 0.5*qt^2
so DVE produces X chunks with fused scalar_tensor_tensor; GPSIMD (which
cannot run that op) produces its chunks in square form (2 ops), and the
per-chunk W1 scaling (x2 or x1) plus the A-bias [W1.Mn^2 over X-chunks]
are folded host-side.  h1/h2 leaky-relus stay exact on ACT with per-chunk
f32 bias APs.  Engine budget per row (ns): PE ~2134, ACT ~3668, DVE ~3564,
GPSIMD ~3834.
"""

from contextlib import ExitStack

import numpy as np

import concourse.bass as bass
import concourse.tile as tile
from concourse import bacc, mybir
from concourse.bass_utils import run_bass_kernel_spmd

N_CORES = 8
N, M, D, H = 512, 512, 1024, 512
K2 = H // 2  # 256
NL = N // N_CORES  # 64 rows per core
DC, HC, KC, MC = D // 128, H // 128, K2 // 128, M // 128  # 8, 4, 2, 4
NEG_SLOPE = 0.01

F32 = mybir.dt.float32
BF16 = mybir.dt.bfloat16
F8 = mybir.dt.float8e4
AF = mybir.ActivationFunctionType
DR = mybir.MatmulPerfMode.DoubleRow
ALU = mybir.AluOpType

# Per-chunk X producer: 'v' = DVE fused X-form, 'p' = GPSIMD 2-op square
# form, 'a' = ACT fused square.  X-form chunks get 2*W1 stationary and
# their W1.Mn^2 term folded into the h1 bias.
X_SPLIT = "vvvvvvpp"

# f32 packed tensor column offsets
_MT0 = 0                      # mt [128, DC, NL] f32 (= -M rows, scalars)
_AP0 = _MT0 + DC * NL         # apb [128, HC, NL]
_B20 = _AP0 + HC * NL         # b2c [128, KC]
_C00 = _B20 + KC              # -b3 column [128, 1]
_MN0 = _C00 + 1               # row0: ||Mn||^2 [1, NL]
_FG0 = _MN0 + NL              # row0: fg mask tiled [1, MC*NL]
_ON0 = _FG0 + MC * NL         # row0: ones [1, 128]
_PA_COLS = _ON0 + 128

# bf16 packed: qt | qh | mtc2
_QT0 = 0
_QH0 = _QT0 + DC * M          # 4096
_MC0 = _QH0 + DC * M          # 8192
_PQ_COLS = _MC0 + DC * NL     # 8704

# fp8 packed: w1 | w2 | w3
_W10 = 0
_W20 = _W10 + DC * H          # 4096
_W30 = _W20 + HC * K2         # 5120
_P8_COLS = _W30 + KC          # 5122


def emit_body(nc, tc, ctx, pa_sb, pq_sb, p8_sb, stats, x_split=X_SPLIT):
    def pool(name, bufs, space="SBUF"):
        return ctx.enter_context(tc.tile_pool(name=name, bufs=bufs, space=space))

    diffp = pool("diffp", 2)
    tvp = pool("tvp", 2)
    h1p = pool("h1p", 2)
    h2p = pool("h2p", 2)
    ep = pool("ep", 1)
    l1ps = pool("l1ps", 4, "PSUM")
    l2ps = pool("l2ps", 1, "PSUM")
    l3ps = pool("l3ps", 1, "PSUM")
    gps = pool("gps", 1, "PSUM")
    consts = pool("km_consts", 1)

    mt_sb = pa_sb[:, _MT0:_AP0].rearrange("p (c n) -> p c n", c=DC)
    apb_sb = pa_sb[:, _AP0:_B20].rearrange("p (c n) -> p c n", c=HC)
    b2c_sb = pa_sb[:, _B20:_C00]
    c0_sb = pa_sb[:, _C00:_C00 + 1]
    mn2h = pa_sb[0:1, _MN0:_MN0 + NL]
    fg_row = pa_sb[0:1, _FG0:_FG0 + MC * NL]
    onesr = pa_sb[0:1, _ON0:_ON0 + 128]
    qt_sb = pq_sb[:, _QT0:_QH0].rearrange("p (c m) -> p c m", c=DC)
    qh_sb = pq_sb[:, _QH0:_MC0].rearrange("p (c m) -> p c m", c=DC)
    mtc2 = pq_sb[:, _MC0:_PQ_COLS].rearrange("p (c n) -> p c n", c=DC)
    w1_sb = p8_sb[:, _W10:_W20].rearrange("p (c h) -> p c h", c=DC)
    w2_sb = p8_sb[:, _W20:_W30].rearrange("p (c k) -> p c k", c=HC)
    w3p = p8_sb[:, _W30:_P8_COLS].rearrange("p (c o) -> p c o", c=KC)

    # ---- prologue: cost[n,m] partial = ||Mn||^2 - 2 Mn.Q  (transposed to
    # [m-part, mc, n]) and fg mask broadcast ----
    g_t = gps.tile([128, MC, NL], F32, tag="g_t")
    for mc in range(MC):
        for dc in range(DC):
            nc.tensor.matmul(g_t[:, mc, :],
                             qt_sb[:, dc, mc * 128:(mc + 1) * 128],
                             mtc2[:, dc, :],
                             start=(dc == 0), stop=False)
        nc.tensor.matmul(g_t[:, mc, :], onesr, mn2h, start=False, stop=True)
    cost_t = consts.tile([128, MC, NL], F32, tag="cost_t")
    nc.vector.tensor_copy(cost_t[:], g_t[:])

    mask_ps = gps.tile([128, MC, NL], F32, tag="g_t")
    nc.tensor.matmul(mask_ps[:].rearrange("p c n -> p (c n)"), onesr, fg_row,
                     start=True, stop=True)
    mask_bc = consts.tile([128, MC, NL], F32, tag="mask_bc")
    nc.vector.tensor_copy(mask_bc[:], mask_ps[:])

    d_ps = l3ps.tile([128, MC, NL], F32, tag="d_ps")

    # ---- main loop over local rows ----
    for n in range(NL):
        x8 = diffp.tile([128, DC, M], F8, tag="x8")
        for dc in range(DC):
            eng = x_split[dc]
            if eng == "v":
                nc.vector.scalar_tensor_tensor(
                    x8[:, dc, :], qt_sb[:, dc, :], mt_sb[:, dc, n:n + 1],
                    qh_sb[:, dc, :], ALU.mult, ALU.add)
            elif eng == "p":
                tv = tvp.tile([128, M], BF16, tag="tv")
                nc.gpsimd.tensor_scalar_add(tv[:], qt_sb[:, dc, :],
                                            mt_sb[:, dc, n:n + 1])
                nc.gpsimd.tensor_mul(x8[:, dc, :], tv[:], tv[:])
            else:
                nc.scalar.activation(x8[:, dc, :], qt_sb[:, dc, :], AF.Square,
                                     bias=mt_sb[:, dc, n:n + 1])

        h1 = h1p.tile([128, HC, M], F8, tag="h1")
        for hc in range(HC):
            p1 = l1ps.tile([128, M], F32, tag="p1")
            for dp in range(DC // 2):
                nc.tensor.matmul(
                    p1[:],
                    w1_sb[:, 2 * dp:2 * dp + 2, hc * 128:(hc + 1) * 128],
                    x8[:, 2 * dp:2 * dp + 2, :],
                    start=(dp == 0), stop=(dp == DC // 2 - 1), perf_mode=DR)
            nc.scalar.activation(h1[:, hc, :], p1[:], AF.Lrelu,
                                 bias=apb_sb[:, hc, n:n + 1], alpha=NEG_SLOPE)

        p2 = l2ps.tile([128, KC, M], F32, tag="p2")
        for kc in range(KC):
            for hp in range(HC // 2):
                nc.tensor.matmul(
                    p2[:, kc, :],
                    w2_sb[:, 2 * hp:2 * hp + 2, kc * 128:(kc + 1) * 128],
                    h1[:, 2 * hp:2 * hp + 2, :],
                    start=(hp == 0), stop=(hp == HC // 2 - 1), perf_mode=DR)
        h2 = h2p.tile([128, KC, M], F8, tag="h2")
        for kc in range(KC):
            nc.scalar.activation(h2[:, kc, :], p2[:, kc, :], AF.Lrelu,
                                 bias=b2c_sb[:, kc:kc + 1], alpha=NEG_SLOPE)

        for mc in range(MC):
            nc.tensor.matmul(
                d_ps[:, mc, n:n + 1],
                h2[:, 0:2, mc * 128:(mc + 1) * 128],
                w3p[:, 0:2, :],
                start=True, stop=True, perf_mode=DR)

    # ---- epilogue ----
    e_t = ep.tile([128, MC, NL], F32, tag="e_t")
    nc.scalar.activation(e_t[:], d_ps[:], AF.Exp, bias=c0_sb, scale=-1.0)
    w_t = ep.tile([128, MC, NL], F32, tag="w_t")
    nc.vector.tensor_mul(w_t[:], e_t[:], cost_t[:])
    efg_t = ep.tile([128, MC, NL], F32, tag="efg_t")
    nc.vector.tensor_mul(efg_t[:], e_t[:], mask_bc[:])
    wfg_t = ep.tile([128, MC, NL], F32, tag="wfg_t")
    nc.vector.tensor_mul(wfg_t[:], w_t[:], mask_bc[:])

    stats_sb = consts.tile([128, 4, MC], F32, tag="stats_sb")
    for s, src in ((0, e_t), (1, efg_t), (2, w_t), (3, wfg_t)):
        nc.vector.tensor_reduce(stats_sb[:, s, :], src[:],
                                axis=mybir.AxisListType.X,
                                op=mybir.AluOpType.add)
    nc.sync.dma_start(stats[:], stats_sb[:])


def build_program(x_split=X_SPLIT):
    nc = bacc.Bacc("TRN2", target_bir_lowering=False, debug=False,
                   num_devices=N_CORES)
    pa = nc.dram_tensor("pa", [128, _PA_COLS], F32, kind="ExternalInput").ap()
    pq = nc.dram_tensor("pq", [128, _PQ_COLS], BF16, kind="ExternalInput").ap()
    p8 = nc.dram_tensor("p8", [128, _P8_COLS], F8, kind="ExternalInput").ap()
    stats = nc.dram_tensor("stats", [128, 4, MC], F32,
                           kind="ExternalOutput").ap()

    with tile.TileContext(nc) as tc, ExitStack() as ctx:
        consts = ctx.enter_context(tc.tile_pool(name="consts", bufs=1))
        pa_sb = consts.tile([128, _PA_COLS], F32, tag="pa_sb")
        nc.sync.dma_start(pa_sb[:], pa[:])
        p8_sb = consts.tile([128, _P8_COLS], F8, tag="p8_sb")
        nc.sync.dma_start(p8_sb[:], p8[:])
        pq_sb = consts.tile([128, _PQ_COLS], BF16, tag="pq_sb")
        nc.sync.dma_start(pq_sb[:, 0:_QH0], pq[:, 0:_QH0])
        nc.sync.dma_start(pq_sb[:, _QH0:_PQ_COLS], pq[:, _QH0:_PQ_COLS])
        emit_body(nc, tc, ctx, pa_sb, pq_sb, p8_sb, stats, x_split)

    nc.compile()
    return nc


def shard_inputs(M_aug, Q, W1, b1, W2, b2, W3, b3, N_fg, x_split=X_SPLIT):
    """Host-side layout prep. Returns per-core input maps."""
    import ml_dtypes
    f = np.float32
    bf = ml_dtypes.bfloat16
    e4 = ml_dtypes.float8_e4m3
    M_aug = np.asarray(M_aug, np.float64)
    Q = np.asarray(Q, np.float64)
    W1 = np.asarray(W1, np.float64)
    W2 = np.asarray(W2, np.float64)
    W3 = np.asarray(W3, np.float64)
    b1 = np.asarray(b1, np.float64)
    b2 = np.asarray(b2, np.float64)
    b3 = np.asarray(b3, np.float64)
    nfg = int(N_fg)

    def part_major(a2d, chunks):  # [C*128, F] -> [128, C*F]
        cdim, fdim = a2d.shape
        assert cdim == chunks * 128
        return np.ascontiguousarray(
            a2d.reshape(chunks, 128, fdim).transpose(1, 0, 2)).reshape(128, -1)

    # which d-chunks use the X-form (scaled 2*W1, A-bias)
    v_chunks = np.array([c == "v" for c in x_split])
    w1_scale = np.where(np.repeat(v_chunks, 128), 2.0, 1.0)  # [D]
    v_dmask = np.repeat(v_chunks, 128).astype(np.float64)     # [D]

    p8_v = np.zeros((128, _P8_COLS), e4)
    p8_v[:, _W10:_W20] = part_major((W1 * w1_scale[None, :]).T, DC).astype(e4)
    p8_v[:, _W20:_W30] = part_major(W2.T, HC).astype(e4)
    p8_v[:, _W30:_P8_COLS] = W3.reshape(KC, 128).T.astype(e4)
    p8_v = np.ascontiguousarray(p8_v)

    qt = part_major(Q.T, DC).astype(bf)                  # [128, DC*M] bf16
    qh = (0.5 * qt.astype(np.float64) ** 2).astype(bf)

    base = np.zeros((128, _PA_COLS), f)
    base[:, _B20:_C00] = b2.reshape(KC, 128).T
    base[:, _C00] = -float(b3[0])
    base[0, _ON0:_ON0 + 128] = 1.0

    in_maps = []
    for c in range(N_CORES):
        rows = slice(c * NL, (c + 1) * NL)
        Mrows = M_aug[rows]                               # [NL, D]
        pa_v = base.copy()
        pa_v[:, _MT0:_AP0] = part_major(-Mrows.T, DC)
        # A' bias: b1 + sum_{d in v-chunks} W1[h,d]*M[n,d]^2, [H, NL]
        Ap = b1[:, None] + W1 @ (v_dmask[:, None] * (Mrows.T ** 2))
        pa_v[:, _AP0:_B20] = part_major(Ap.astype(f), HC)
        pa_v[0, _MN0:_MN0 + NL] = (Mrows ** 2).sum(-1)
        gidx = np.arange(c * NL, (c + 1) * NL)
        pa_v[0, _FG0:_FG0 + MC * NL] = np.tile((gidx < nfg).astype(f), MC)

        pq_v = np.zeros((128, _PQ_COLS), bf)
        pq_v[:, _QT0:_QH0] = qt
        pq_v[:, _QH0:_MC0] = qh
        pq_v[:, _MC0:_PQ_COLS] = part_major(-2.0 * Mrows.T, DC).astype(bf)
        in_maps.append({"pa": pa_v, "pq": np.ascontiguousarray(pq_v),
                        "p8": p8_v})
    return in_maps


def combine(stats_list, Q):
    """stats_list: per-core [128, 4, MC] arrays -> (score, score_fg)."""
    st = np.stack([
        np.asarray(s, np.float64).transpose(1, 2, 0).reshape(4, M)
        for s in stats_list
    ])  # [C, 4, M]
    S1 = st[:, 0].sum(0)
    S1fg = st[:, 1].sum(0)
    Sc = st[:, 2].sum(0)
    Scfg = st[:, 3].sum(0)
    qn2 = (np.asarray(Q, np.float64) ** 2).sum(-1)
    score = Sc / S1 + qn2
    score_fg = Scfg / S1 + qn2 * (S1fg / S1)
    return score.astype(np.float32), score_fg.astype(np.float32)


_PROGRAM_CACHE = {}


def run(trace=False, **inputs):
    if "prog" not in _PROGRAM_CACHE:
        _PROGRAM_CACHE["prog"] = build_program()
    nc = _PROGRAM_CACHE["prog"]
    in_maps = shard_inputs(**inputs)
    res = run_bass_kernel_spmd(nc, in_maps, list(range(N_CORES)), trace=trace)
    outs = combine([res.results[c]["stats"] for c in range(N_CORES)],
                   inputs["Q"])
    return outs, res


def kernel(**inputs):
    outs, _ = run(trace=False, **inputs)
    return outs
